# revision 1
# baseline (speedup 1.0000x reference)
"""Self-contained Trainium2 Bass kernel for nn_Attention (8-head self-attention).

Reference computation (per batch element b):
    xt = x[b].reshape(C, N).T            # (N, C),  N = H*W = 1024
    q  = xt @ Wq                         # (N, 512)
    k, v = split(xt @ Wkv)               # (N, 512) each
    per head h (d=64): sim = q_h k_h^T / 8 ; P = softmax(sim) ; o_h = P v_h
    out[b] = concat_h(o_h) @ Wo + bo     # (N, C)

Sharding: pure data parallel -- core b computes batch element b (8 cores, 8
batch elements, no collectives).

Layout strategy (keeps every matmul contraction dim on partitions, zero
on-chip transposes):
  - x[b] is used as (C, N): already the transpose of xt.
  - qT, kT are produced in (inner, N) layout; V in (N, inner) layout with an
    extra ones-column per head so the P@V matmul also emits the softmax
    denominators (M = 64+1 = 65).
  - simT tiles are (key j on partitions, query i on free); exp runs on
    ScalarE straight out of PSUM into bf16 SBUF. Attention steady state is
    paced by the 8 exp ops per head (~1.3us each).
  - Softmax denominators: bounce through DRAM in contiguous 8-elem chunks to
    spread the row across 128 partitions, reciprocal there, then a 0-stride
    DMA replicates 1/s for the normalize multiply. The last attn@v matmul +
    PSUM readout of head h fire early in head h+1 and the recip+mul a head
    after that, so chain latency never blocks the in-order PE/DVE queues.
  - PSUM budget (8 banks): 4 for sim/projection tiles (shared tag, double
    buffered) + 4 for two in-flight attn@v accumulators.
"""

import numpy as np

import concourse.bass as bass
import concourse.mybir as mybir
import concourse.tile as tile
from concourse import bacc

B, C, N = 8, 512, 1024
HEADS, D = 8, 64
INNER = HEADS * D  # 512
SCALE = D ** -0.5
P = 128
CT = C // P       # 4  k-tiles over C
MT = INNER // P   # 4  partition-tiles over inner
JT = N // P       # 8  key tiles
NT = N // P       # 8  output row tiles
NB = N // 512     # 2  free-dim blocks of 512 over N

F32 = mybir.dt.float32
BF16 = mybir.dt.bfloat16
EXP = mybir.ActivationFunctionType.Exp


def build_nc(debug=False):
    nc = bacc.Bacc(
        "TRN2", target_bir_lowering=False, debug=debug, num_devices=B
    )
    x_d = nc.dram_tensor("x", [C, N], F32, kind="ExternalInput")
    wq_d = nc.dram_tensor("Wq", [C, INNER], F32, kind="ExternalInput")
    wkv_d = nc.dram_tensor("Wkv", [C, 2 * INNER], F32, kind="ExternalInput")
    wo_d = nc.dram_tensor("Wo", [INNER, C], F32, kind="ExternalInput")
    bo_d = nc.dram_tensor("bo", [C], F32, kind="ExternalInput")
    out_d = nc.dram_tensor("out", [N, C], F32, kind="ExternalOutput")

    with tile.TileContext(nc) as tc:
        with (
            tc.tile_pool(name="persist", bufs=1) as persist,
            tc.tile_pool(name="stage", bufs=1) as stage,
            tc.tile_pool(name="etp", bufs=3) as etp,
            tc.tile_pool(name="ovp", bufs=6) as ovp,
            tc.tile_pool(name="small", bufs=4) as small,
            tc.tile_pool(name="dramp", bufs=2, space="DRAM") as dramp,
            tc.tile_pool(name="psS", bufs=2, space="PSUM") as psS,
            tc.tile_pool(name="psO", bufs=2, space="PSUM") as psO,
        ):
            # ---------------- load + cast inputs ----------------
            # x / Wq / Wkv are loaded+cast per 128-row chunk so the first
            # projection matmuls start as soon as their chunks land.
            x_f = stage.tile([P, CT, N], F32, tag="st_x")
            x_b = persist.tile([P, CT, N], BF16)
            x_dv = x_d[:].rearrange("(a p) n -> p a n", p=P)
            wq_f = stage.tile([P, CT, INNER], F32, tag="st_q")
            wq_b = persist.tile([P, CT, INNER], BF16)
            wq_dv = wq_d[:].rearrange("(a p) m -> p a m", p=P)
            wkv_f = stage.tile([P, CT, 2 * INNER], F32, tag="st_kv")
            wkv_b = persist.tile([P, CT, 2 * INNER], BF16)
            wkv_dv = wkv_d[:].rearrange("(a p) m -> p a m", p=P)
            for a in range(CT):
                nc.sync.dma_start(out=x_f[:, a, :], in_=x_dv[:, a, :])
                nc.vector.tensor_copy(out=x_b[:, a, :], in_=x_f[:, a, :])
                nc.sync.dma_start(out=wq_f[:, a, :], in_=wq_dv[:, a, :])
                nc.scalar.copy(out=wq_b[:, a, :], in_=wq_f[:, a, :])
            # Wkv is first needed by k-proj, which runs after all of q-proj;
            # loading it after x/Wq tightens the startup ramp
            for a in range(CT):
                nc.sync.dma_start(out=wkv_f[:, a, :], in_=wkv_dv[:, a, :])
                nc.scalar.copy(out=wkv_b[:, a, :], in_=wkv_f[:, a, :])

            wo_f = stage.tile([P, MT, C], F32, tag="st_q")
            nc.sync.dma_start(out=wo_f, in_=wo_d[:].rearrange("(a p) m -> p a m", p=P))
            wo_b = persist.tile([P, MT, C], BF16)
            nc.vector.tensor_copy(out=wo_b, in_=wo_f)

            bo_bc = persist.tile([P, C], F32)
            bo_ap = bo_d[:]
            nc.gpsimd.dma_start(
                out=bo_bc,
                in_=bass.AP(tensor=bo_ap.tensor, offset=bo_ap.offset,
                            ap=[[0, P], [1, C]]),
            )

            zb = persist.tile([P, 1], F32)
            nc.vector.memset(zb, 0.0)

            # ---------------- projections ----------------
            # qT, kT: (inner, N) transposed layout; inner = mt*128 + p.
            # Order: k/q for mt=0 first (gates head 0), then V (gates the
            # first attn@v), then the remaining k/q tiles.
            qT = persist.tile([P, MT, N], BF16)
            kT = persist.tile([P, MT, N], BF16)
            v_ext = persist.tile([P, JT, HEADS, D + 1], BF16)
            nc.vector.memset(v_ext[:, :, :, D], 1.0)

            def kq_proj(mt):
                for dst, w_b in ((kT, wkv_b), (qT, wq_b)):
                    for ib in range(NB):
                        psf = psS.tile([P, N], F32, tag="st")
                        ps = psf[:, 0:512]
                        for a in range(CT):
                            nc.tensor.matmul(
                                ps,
                                lhsT=w_b[:, a, mt * P:(mt + 1) * P],
                                rhs=x_b[:, a, ib * 512:(ib + 1) * 512],
                                start=(a == 0),
                                stop=(a == CT - 1),
                            )
                        nc.vector.tensor_copy(
                            out=dst[:, mt, ib * 512:(ib + 1) * 512], in_=ps)

            def v_proj():
                # V in normal layout (token j on partitions), per head with an
                # extra ones column: v_ext[:, jt, h, 0:64] = V, [..., 64] = 1
                for jt in range(JT):
                    psf = psS.tile([P, N], F32, tag="st")
                    ps = psf[:, 0:512]
                    for a in range(CT):
                        nc.tensor.matmul(
                            ps,
                            lhsT=x_b[:, a, jt * P:(jt + 1) * P],
                            rhs=wkv_b[:, a, INNER:2 * INNER],
                            start=(a == 0),
                            stop=(a == CT - 1),
                        )
                    nc.vector.tensor_copy(
                        out=v_ext[:, jt, :, 0:D],
                        in_=ps.rearrange("p (h d) -> p h d", h=HEADS),
                    )

            # dense projection phase first measures fastest overall: the PE
            # runs it uninterrupted while the attention pipeline spins up
            for mt in range(MT):
                kq_proj(mt)
            v_proj()

            # ---------------- attention (per head) ----------------
            # normalized O^T as one tile PER head-pair: gives the output
            # projection per-pair dependencies, so its kk<3 matmuls can
            # run while the last head's denominator chain drains
            oTs = []
            for m in range(MT):
                oT_m = persist.tile([P, N], BF16, tag=f"oT{m}")
                oTs.append(oT_m)
            pending_avtail = None   # emits av-tail of h-1, returns finish
            pending_finish = None   # finish of h-2
            for h in range(HEADS):
                hp = (h % 2) * D
                hm = h // 2
                qh = qT[hp:hp + D, hm, :]   # [64, N]
                kh = kT[hp:hp + D, hm, :]   # [64, N]

                et = etp.tile([P, JT, N], BF16, tag="et")
                for jt in range(JT):
                    st = psS.tile([P, N], F32, tag="st")
                    for ib in range(NB):
                        nc.tensor.matmul(
                            st[:, ib * 512:(ib + 1) * 512],
                            lhsT=kh[:, jt * P:(jt + 1) * P],
                            rhs=qh[:, ib * 512:(ib + 1) * 512],
                            start=True,
                            stop=True,
                        )
                    # E^T = exp(scale * S^T), PSUM -> bf16 SBUF
                    nc.scalar.activation(
                        out=et[:, jt, :], in_=st, func=EXP, bias=zb, scale=SCALE)
                    if jt == 1:
                        # previous head's attn@v tail + epilogue fire here so
                        # this head's first sim groups keep the exp stream
                        # bubble-free across the head boundary; the finish
                        # (recip+mul) of the head before that fires too
                        if pending_finish is not None:
                            pending_finish()
                            pending_finish = None
                        if pending_avtail is not None:
                            pending_finish = pending_avtail()
                            pending_avtail = None

                # O'^T_ext = [V_h | 1]^T @ E^T ; row D is the softmax denom
                # (last key tile + readout deferred into the next head)
                ov = psO.tile([D + 1, N], F32, tag="ov")
                for jt in range(JT - 1):
                    for ib in range(NB):
                        nc.tensor.matmul(
                            ov[:, ib * 512:(ib + 1) * 512],
                            lhsT=v_ext[:, jt, h, :],
                            rhs=et[:, jt, ib * 512:(ib + 1) * 512],
                            start=(jt == 0),
                            stop=False,
                        )

                def avtail(h=h, ov=ov, et=et):
                    jt = JT - 1
                    for ib in range(NB):
                        nc.tensor.matmul(
                            ov[:, ib * 512:(ib + 1) * 512],
                            lhsT=v_ext[:, jt, h, :],
                            rhs=et[:, jt, ib * 512:(ib + 1) * 512],
                            start=False,
                            stop=True,
                        )
                    # two quick copies release the PSUM tile; the s-row copy
                    # goes first since it gates the recip chain
                    s_tmp = small.tile([1, N], F32, tag="stmp")
                    nc.vector.tensor_copy(out=s_tmp, in_=ov[D:D + 1, :])
                    ov_sb = ovp.tile([D, N], BF16, tag="ovsb")
                    nc.vector.tensor_copy(out=ov_sb, in_=ov[0:D, :])
                    # 1/denom 128 lanes wide: bounce through DRAM to spread
                    # the row across partitions (contiguous 8-elem chunks)
                    sd = dramp.tile([N], F32, tag="sd")
                    nc.sync.dma_start(out=sd, in_=s_tmp)
                    st2 = small.tile([P, NT], F32, tag="st2")
                    nc.sync.dma_start(
                        out=st2, in_=sd.rearrange("(p k) -> p k", k=NT))

                    def finish(h=h, ov_sb=ov_sb, st2=st2):
                        rst2 = small.tile([P, NT], F32, tag="rst2")
                        nc.vector.reciprocal(rst2, st2)
                        rsd = dramp.tile([N], F32, tag="rsd")
                        nc.sync.dma_start(
                            out=rsd.rearrange("(p k) -> p k", k=NT), in_=rst2)
                        rep = small.tile([D, N], F32, tag="rep")
                        rsd_ap = rsd[:]
                        nc.sync.dma_start(
                            out=rep,
                            in_=bass.AP(tensor=rsd_ap.tensor,
                                        offset=rsd_ap.offset,
                                        ap=[[0, D], [1, N]]),
                        )
                        hp2 = (h % 2) * D
                        nc.vector.tensor_mul(
                            oTs[h // 2][hp2:hp2 + D, :], ov_sb, rep)

                    return finish

                pending_avtail = avtail

            # drain the deferral chain: finish(6), avtail(7), finish(7)
            if pending_finish is not None:
                pending_finish()
            pending_finish = pending_avtail()
            pending_finish()

            # ---------------- output projection ----------------
            for it in range(NT):
                pff = psS.tile([P, N], F32, tag="st")
                pf = pff[:, 0:C]
                for kk in range(MT):
                    nc.tensor.matmul(
                        pf,
                        lhsT=oTs[kk][:, it * P:(it + 1) * P],
                        rhs=wo_b[:, kk, :],
                        start=(kk == 0),
                        stop=(kk == MT - 1),
                    )
                fin = small.tile([P, C], F32, tag="fin")
                nc.vector.tensor_add(fin, pf, bo_bc)
                nc.sync.dma_start(out=out_d[it * P:(it + 1) * P, :], in_=fin)

    return nc


def kernel(x, Wq, Wkv, Wo, bo):
    from concourse.bass_utils import run_bass_kernel_spmd

    nc = build_nc()
    nc.compile()
    x = np.asarray(x)
    xs = np.ascontiguousarray(x.reshape(B, C, N)).astype(np.float32, copy=False)
    in_maps = [
        {
            "x": xs[b],
            "Wq": np.asarray(Wq, dtype=np.float32),
            "Wkv": np.asarray(Wkv, dtype=np.float32),
            "Wo": np.asarray(Wo, dtype=np.float32),
            "bo": np.asarray(bo, dtype=np.float32),
        }
        for b in range(B)
    ]
    res = run_bass_kernel_spmd(nc, in_maps, list(range(B)))
    return np.stack([res.results[b]["out"] for b in range(B)], axis=0)



# revision 4
# speedup vs baseline: 1.0179x; 1.0179x over previous
"""Self-contained Trainium2 Bass kernel for nn_Attention (8-head self-attention).

Reference computation (per batch element b):
    xt = x[b].reshape(C, N).T            # (N, C),  N = H*W = 1024
    q  = xt @ Wq                         # (N, 512)
    k, v = split(xt @ Wkv)               # (N, 512) each
    per head h (d=64): sim = q_h k_h^T / 8 ; P = softmax(sim) ; o_h = P v_h
    out[b] = concat_h(o_h) @ Wo + bo     # (N, C)

Sharding: pure data parallel -- core b computes batch element b (8 cores, 8
batch elements, no collectives).

Layout strategy (keeps every matmul contraction dim on partitions, zero
on-chip transposes):
  - x[b] is used as (C, N): already the transpose of xt.
  - qT, kT are produced in (inner, N) layout; V in (N, inner) layout with an
    extra ones-column per head so the P@V matmul also emits the softmax
    denominators (M = 64+1 = 65).
  - Heads are processed in PAIRS (2m, 2m+1). Head 2m lives on partitions
    0:64 of qT/kT[:, m, :], head 2m+1 on 64:128, so their K=64 sim matmuls
    occupy disjoint PE row groups and execute CONCURRENTLY (row tiling).
  - exp is split across two engines: ScalarE does the even head with the
    real activation, VectorE does the odd head with a Schraudolph-style
    fast exp (affine in f32, convert to int16 = the bf16 bit pattern of
    exp, bitcast back to bf16). Max rel err ~3% pre-softmax; systematic
    part cancels in the normalization; end-to-end contribution ~3e-3.
  - Attention is software-pipelined: the attn@v matmuls for sim tile jt
    run two slots later (jt+2), the previous pair's epilogue (PSUM drain
    via ScalarE copy, denominator reciprocal via a DRAM scatter bounce,
    bf16 normalize multiply on VectorE) is spread over the next pair's
    early slots.
  - PSUM budget: 2 sim tiles (2 banks each, one per head of the pair) +
    two [65, N] attn@v accumulators (2 banks each, double buffered across
    pairs) = 8 banks.
"""

import numpy as np

import concourse.bass as bass
import concourse.mybir as mybir
import concourse.tile as tile
from concourse import bacc

B, C, N = 8, 512, 1024
HEADS, D = 8, 64
INNER = HEADS * D  # 512
SCALE = D ** -0.5
P = 128
CT = C // P       # 4  k-tiles over C
MT = INNER // P   # 4  partition-tiles over inner
JT = N // P       # 8  key tiles
NT = N // P       # 8  output row tiles
NB = N // 512     # 2  free-dim blocks of 512 over N
PAIRS = HEADS // 2

F32 = mybir.dt.float32
BF16 = mybir.dt.bfloat16
I16 = mybir.dt.int16
EXP = mybir.ActivationFunctionType.Exp
COPY = mybir.ActivationFunctionType.Copy
MULT = mybir.AluOpType.mult
ADD = mybir.AluOpType.add

# fast-exp constants: bf16 bits of exp(s * SCALE) ~= int16(FEXP_A * s + FEXP_B)
FEXP_A = 128.0 * 1.4426950408889634 * SCALE
FEXP_B = 16250.65


def build_nc(debug=False):
    nc = bacc.Bacc(
        "TRN2", target_bir_lowering=False, debug=debug, num_devices=B
    )
    x_d = nc.dram_tensor("x", [C, N], F32, kind="ExternalInput")
    wq_d = nc.dram_tensor("Wq", [C, INNER], F32, kind="ExternalInput")
    wkv_d = nc.dram_tensor("Wkv", [C, 2 * INNER], F32, kind="ExternalInput")
    wo_d = nc.dram_tensor("Wo", [INNER, C], F32, kind="ExternalInput")
    bo_d = nc.dram_tensor("bo", [C], F32, kind="ExternalInput")
    out_d = nc.dram_tensor("out", [N, C], F32, kind="ExternalOutput")

    with tile.TileContext(nc) as tc:
        with (
            tc.tile_pool(name="persist", bufs=1) as persist,
            tc.tile_pool(name="stage", bufs=1) as stage,
            tc.tile_pool(name="etp", bufs=2) as etp,
            tc.tile_pool(name="small", bufs=4) as small,
            tc.tile_pool(name="dramp", bufs=2, space="DRAM") as dramp,
            tc.tile_pool(name="psA", bufs=1, space="PSUM") as psA,
            tc.tile_pool(name="psB", bufs=1, space="PSUM") as psB,
            tc.tile_pool(name="psO", bufs=2, space="PSUM") as psO,
        ):
            # ---------------- load + cast inputs ----------------
            x_f = stage.tile([P, CT, N], F32, tag="st_x")
            x_b = persist.tile([P, CT, N], BF16)
            x_dv = x_d[:].rearrange("(a p) n -> p a n", p=P)
            wq_f = stage.tile([P, CT, INNER], F32, tag="st_q")
            wq_b = persist.tile([P, CT, INNER], BF16)
            wq_dv = wq_d[:].rearrange("(a p) m -> p a m", p=P)
            wkv_f = stage.tile([P, CT, 2 * INNER], F32, tag="st_kv")
            wkv_b = persist.tile([P, CT, 2 * INNER], BF16)
            wkv_dv = wkv_d[:].rearrange("(a p) m -> p a m", p=P)
            for a in range(CT):
                nc.sync.dma_start(out=x_f[:, a, :], in_=x_dv[:, a, :])
                nc.vector.tensor_copy(out=x_b[:, a, :], in_=x_f[:, a, :])
                nc.sync.dma_start(out=wq_f[:, a, :], in_=wq_dv[:, a, :])
                nc.scalar.copy(out=wq_b[:, a, :], in_=wq_f[:, a, :])
            for a in range(CT):
                nc.sync.dma_start(out=wkv_f[:, a, :], in_=wkv_dv[:, a, :])
                nc.scalar.copy(out=wkv_b[:, a, :], in_=wkv_f[:, a, :])

            wo_f = stage.tile([P, MT, C], F32, tag="st_q")
            nc.sync.dma_start(out=wo_f, in_=wo_d[:].rearrange("(a p) m -> p a m", p=P))
            wo_b = persist.tile([P, MT, C], BF16)
            nc.vector.tensor_copy(out=wo_b, in_=wo_f)

            bo_bc = persist.tile([P, C], F32)
            bo_ap = bo_d[:]
            nc.gpsimd.dma_start(
                out=bo_bc,
                in_=bass.AP(tensor=bo_ap.tensor, offset=bo_ap.offset,
                            ap=[[0, P], [1, C]]),
            )

            zb = persist.tile([P, 1], F32)
            nc.vector.memset(zb, 0.0)

            # ---------------- projections ----------------
            qT = persist.tile([P, MT, N], BF16)
            kT = persist.tile([P, MT, N], BF16)
            v_ext = persist.tile([P, JT, HEADS, D + 1], BF16)
            nc.vector.memset(v_ext[:, :, :, D], 1.0)

            def kq_proj(mt):
                # k first (not actually required before q, kept for symmetry)
                for dst, w_b in ((kT, wkv_b), (qT, wq_b)):
                    psf = (psA if dst is kT else psB).tile(
                        [P, N], F32, tag="sim")
                    for ib in range(NB):
                        ps = psf[:, ib * 512:(ib + 1) * 512]
                        for a in range(CT):
                            nc.tensor.matmul(
                                ps,
                                lhsT=w_b[:, a, mt * P:(mt + 1) * P],
                                rhs=x_b[:, a, ib * 512:(ib + 1) * 512],
                                start=(a == 0),
                                stop=(a == CT - 1),
                            )
                    nc.vector.tensor_copy(out=dst[:, mt, :], in_=psf)

            def v_proj():
                # V in normal layout (token j on partitions), per head with an
                # extra ones column: v_ext[:, jt, h, 0:64] = V, [..., 64] = 1
                for jt2 in range(JT // 2):
                    psf = (psA if jt2 % 2 == 0 else psB).tile(
                        [P, N], F32, tag="sim")
                    for half in range(2):
                        jt = jt2 * 2 + half
                        ps = psf[:, half * 512:(half + 1) * 512]
                        for a in range(CT):
                            nc.tensor.matmul(
                                ps,
                                lhsT=x_b[:, a, jt * P:(jt + 1) * P],
                                rhs=wkv_b[:, a, INNER:2 * INNER],
                                start=(a == 0),
                                stop=(a == CT - 1),
                            )
                        nc.vector.tensor_copy(
                            out=v_ext[:, jt, :, 0:D],
                            in_=ps.rearrange("p (h d) -> p h d", h=HEADS),
                        )

            for mt in range(MT):
                kq_proj(mt)
            v_proj()

            # ---------------- attention (pair-pipelined) ----------------
            oTs = []
            for m in range(PAIRS):
                oT_m = persist.tile([P, N], BF16, tag=f"oT{m}")
                oTs.append(oT_m)

            et_tiles = {}
            ov_tiles = {}

            def emit_sim(m, jt):
                """Concurrent row-tiled sims for heads 2m (rows 0:64) and
                2m+1 (rows 64:128), then exp on ScalarE / fast-exp on DVE."""
                st0 = psA.tile([P, N], F32, tag="sim")
                st1 = psB.tile([P, N], F32, tag="sim")
                et = et_tiles[m]
                for ib in range(NB):
                    sl = slice(ib * 512, (ib + 1) * 512)
                    nc.tensor.matmul(
                        st0[:, sl],
                        lhsT=kT[0:D, m, jt * P:(jt + 1) * P],
                        rhs=qT[0:D, m, sl],
                        start=True, stop=True,
                    )
                    nc.tensor.matmul(
                        st1[:, sl],
                        lhsT=kT[D:P, m, jt * P:(jt + 1) * P],
                        rhs=qT[D:P, m, sl],
                        start=True, stop=True,
                    )
                # even head: real exp on ScalarE (f32 PSUM -> bf16 SBUF)
                nc.scalar.activation(
                    out=et[:, 0, jt, :], in_=st0, func=EXP, bias=zb,
                    scale=SCALE)
                # odd head: fast exp on VectorE (affine -> int16 = bf16 bits)
                nc.vector.tensor_scalar(
                    et[:, 1, jt, :].bitcast(I16),
                    st1,
                    FEXP_A,
                    FEXP_B,
                    MULT,
                    ADD,
                )

            def emit_av(m, jt):
                """attn@v chunk for sim tile jt of pair m (4 matmuls)."""
                et = et_tiles[m]
                for hh in range(2):
                    ov = ov_tiles[(m, hh)]
                    for ib in range(NB):
                        sl = slice(ib * 512, (ib + 1) * 512)
                        nc.tensor.matmul(
                            ov[:, sl],
                            lhsT=v_ext[:, jt, 2 * m + hh, :],
                            rhs=et[:, hh, jt, sl],
                            start=(jt == 0),
                            stop=(jt == JT - 1),
                        )

            def emit_drain(m, hh):
                """ScalarE drains ov (o rows + denom row) to bf16 SBUF and
                kicks off the denominator reciprocal bounce."""
                ov = ov_tiles.pop((m, hh))
                ov_sb = small.tile([D + 1, N], BF16, tag="ovsb")
                nc.scalar.activation(out=ov_sb, in_=ov, func=COPY)
                # denom row -> DRAM -> [128, 8] scatter -> reciprocal
                sd = dramp.tile([N], BF16, tag="sd")
                nc.sync.dma_start(out=sd, in_=ov_sb[D:D + 1, :])
                st2 = small.tile([P, NT], BF16, tag="st2")
                nc.sync.dma_start(
                    out=st2, in_=sd.rearrange("(p k) -> p k", k=NT))
                rst2 = small.tile([P, NT], BF16, tag="rst2")
                with nc.allow_low_precision(
                        reason="softmax denom reciprocal in bf16; "
                        "~0.4% rel err, within the 2e-2 budget"):
                    nc.vector.reciprocal(rst2, st2)
                rsd = dramp.tile([N], BF16, tag="rsd")
                nc.sync.dma_start(
                    out=rsd.rearrange("(p k) -> p k", k=NT), in_=rst2)
                rep = small.tile([D, N], BF16, tag="rep")
                rsd_ap = rsd[:]
                nc.sync.dma_start(
                    out=rep,
                    in_=bass.AP(tensor=rsd_ap.tensor,
                                offset=rsd_ap.offset,
                                ap=[[0, D], [1, N]]),
                )
                return ov_sb, rep

            def emit_norm(m, hh, ov_sb, rep):
                hp = hh * D
                nc.vector.tensor_mul(
                    oTs[m][hp:hp + D, :], ov_sb[0:D, :], rep)

            # deferred-work schedule per slot (m, jt):
            #   jt>=2       : attn@v chunk (m, jt-2)
            #   jt=0,1      : attn@v chunks (m-1, 6) and (m-1, 7)
            #   jt=2,3      : drains of pair m-1 (hh=0,1)
            #   jt=4,5      : norms of pair m-1
            pending_drain = {}
            for m in range(PAIRS):
                et_tiles[m] = etp.tile([P, 2, JT, N], BF16, tag="et", name=f"et{m}")
                for hh in range(2):
                    ov_tiles[(m, hh)] = psO.tile([D + 1, N], F32, tag="ov", name=f"ov{m}_{hh}")
                for jt in range(JT):
                    emit_sim(m, jt)
                    if jt >= 2:
                        emit_av(m, jt - 2)
                    elif m > 0:
                        emit_av(m - 1, JT - 2 + jt)
                    if m > 0:
                        if jt in (2, 3):
                            hh = jt - 2
                            pending_drain[hh] = emit_drain(m - 1, hh)
                        elif jt in (4, 5):
                            hh = jt - 4
                            emit_norm(m - 1, hh, *pending_drain.pop(hh))
            # tail: last pair's remaining chunks + epilogue
            m = PAIRS - 1
            emit_av(m, JT - 2)
            emit_av(m, JT - 1)
            d0 = emit_drain(m, 0)
            d1 = emit_drain(m, 1)
            emit_norm(m, 0, *d0)
            emit_norm(m, 1, *d1)

            # ---------------- output projection ----------------
            for it in range(NT):
                psf = (psA if it % 2 == 0 else psB).tile([P, N], F32, tag="sim")
                pf = psf[:, 0:C]
                for kk in range(MT):
                    nc.tensor.matmul(
                        pf,
                        lhsT=oTs[kk][:, it * P:(it + 1) * P],
                        rhs=wo_b[:, kk, :],
                        start=(kk == 0),
                        stop=(kk == MT - 1),
                    )
                fin = small.tile([P, C], F32, tag="fin")
                nc.vector.tensor_add(fin, pf, bo_bc)
                nc.sync.dma_start(out=out_d[it * P:(it + 1) * P, :], in_=fin)

    return nc


def kernel(x, Wq, Wkv, Wo, bo):
    from concourse.bass_utils import run_bass_kernel_spmd

    nc = build_nc()
    nc.compile()
    x = np.asarray(x)
    xs = np.ascontiguousarray(x.reshape(B, C, N)).astype(np.float32, copy=False)
    in_maps = [
        {
            "x": xs[b],
            "Wq": np.asarray(Wq, dtype=np.float32),
            "Wkv": np.asarray(Wkv, dtype=np.float32),
            "Wo": np.asarray(Wo, dtype=np.float32),
            "bo": np.asarray(bo, dtype=np.float32),
        }
        for b in range(B)
    ]
    res = run_bass_kernel_spmd(nc, in_maps, list(range(B)))
    return np.stack([res.results[b]["out"] for b in range(B)], axis=0)


# revision 6
# speedup vs baseline: 1.0253x; 1.0073x over previous
"""Self-contained Trainium2 Bass kernel for nn_Attention (8-head self-attention).

Reference computation (per batch element b):
    xt = x[b].reshape(C, N).T            # (N, C),  N = H*W = 1024
    q  = xt @ Wq                         # (N, 512)
    k, v = split(xt @ Wkv)               # (N, 512) each
    per head h (d=64): sim = q_h k_h^T / 8 ; P = softmax(sim) ; o_h = P v_h
    out[b] = concat_h(o_h) @ Wo + bo     # (N, C)

Sharding: pure data parallel -- core b computes batch element b (8 cores, 8
batch elements, no collectives).

Layout strategy (keeps every matmul contraction dim on partitions, zero
on-chip transposes):
  - x[b] is used as (C, N): already the transpose of xt.
  - qT, kT are produced in (inner, N) layout; V in (N, inner) layout with an
    extra ones-column per head so the P@V matmul also emits the softmax
    denominators (M = 64+1 = 65).
  - Heads are processed in PAIRS (2m, 2m+1). Head 2m lives on partitions
    0:64 of qT/kT[:, m, :], head 2m+1 on 64:128, so their K=64 sim matmuls
    occupy disjoint PE row groups and execute CONCURRENTLY (row tiling).
  - Sim tiles are QUERY-HALVED: [128 keys, 512 queries] = one PSUM bank,
    so each exp consumer gets a double-buffered chain (psA/psB bufs=2) and
    the sim -> exp -> sim PSUM-reuse chain never serializes the slot.
  - exp is split across two engines: ScalarE runs the real activation on
    two of the four (head, qhalf) tiles per key-tile step, VectorE runs a
    Schraudolph-style fast exp (affine in f32, convert to int16 = the bf16
    bit pattern of exp, bitcast back to bf16) on the other two. Fast-exp
    max rel err ~3% pre-softmax; the systematic part cancels in the
    normalization; end-to-end contribution ~3e-3.
  - attn@v uses F=1024 bf16 moving operands (one matmul per (head, key
    tile), output spans 2 PSUM banks).
  - Pipelining: attn@v for key tile jt runs at slot jt+2; the previous
    pair's epilogue (ScalarE PSUM drain, denominator reciprocal via a
    DRAM scatter bounce, bf16 normalize multiply on VectorE) occupies the
    next pair's early slots.
  - HAM warmup: dummy matmuls paced by the input-load casts keep the PE
    activity monitor from idling the clock to 1.2 GHz before/through the
    compute phases.
  - PSUM budget: 4 sim banks + two [65, N] attn@v accumulators (2 banks
    each, double buffered across pairs) = 8 banks.
"""

import numpy as np

import concourse.bass as bass
import concourse.mybir as mybir
import concourse.tile as tile
from concourse import bacc

B, C, N = 8, 512, 1024
HEADS, D = 8, 64
INNER = HEADS * D  # 512
SCALE = D ** -0.5
P = 128
CT = C // P       # 4  k-tiles over C
MT = INNER // P   # 4  partition-tiles over inner
JT = N // P       # 8  key tiles
NT = N // P       # 8  output row tiles
NB = N // 512     # 2  free-dim blocks of 512 over N
PAIRS = HEADS // 2

F32 = mybir.dt.float32
BF16 = mybir.dt.bfloat16
I16 = mybir.dt.int16
EXP = mybir.ActivationFunctionType.Exp
COPY = mybir.ActivationFunctionType.Copy
MULT = mybir.AluOpType.mult
ADD = mybir.AluOpType.add

# fast-exp constants: bf16 bits of exp(s * SCALE) ~= int16(FEXP_A * s + FEXP_B)
FEXP_A = 128.0 * 1.4426950408889634 * SCALE
FEXP_B = 16250.65

AV_F = 512  # attn@v free size (matmul output cannot cross a PSUM bank)


def build_nc(debug=False):
    nc = bacc.Bacc(
        "TRN2", target_bir_lowering=False, debug=debug, num_devices=B
    )
    x_d = nc.dram_tensor("x", [C, N], F32, kind="ExternalInput")
    wq_d = nc.dram_tensor("Wq", [C, INNER], F32, kind="ExternalInput")
    wkv_d = nc.dram_tensor("Wkv", [C, 2 * INNER], F32, kind="ExternalInput")
    wo_d = nc.dram_tensor("Wo", [INNER, C], F32, kind="ExternalInput")
    bo_d = nc.dram_tensor("bo", [C], F32, kind="ExternalInput")
    out_d = nc.dram_tensor("out", [N, C], F32, kind="ExternalOutput")

    with tile.TileContext(nc) as tc:
        with (
            tc.tile_pool(name="persist", bufs=1) as persist,
            tc.tile_pool(name="stage", bufs=1) as stage,
            tc.tile_pool(name="etp", bufs=2) as etp,
            tc.tile_pool(name="small", bufs=4) as small,
            tc.tile_pool(name="dramp", bufs=2, space="DRAM") as dramp,
            tc.tile_pool(name="psA", bufs=2, space="PSUM") as psA,
            tc.tile_pool(name="psB", bufs=2, space="PSUM") as psB,
            tc.tile_pool(name="psO", bufs=2, space="PSUM") as psO,
        ):
            # ---------------- load + cast inputs ----------------
            # x + wkv on the sync DMA queue, wq + wo + bo on the gpsimd
            # queue so the transfers overlap.
            x_f = stage.tile([P, CT, N], F32, tag="st_x")
            x_b = persist.tile([P, CT, N], BF16)
            x_dv = x_d[:].rearrange("(a p) n -> p a n", p=P)
            wq_f = stage.tile([P, CT, INNER], F32, tag="st_q")
            wq_b = persist.tile([P, CT, INNER], BF16)
            wq_dv = wq_d[:].rearrange("(a p) m -> p a m", p=P)
            wkv_f = stage.tile([P, CT, 2 * INNER], F32, tag="st_kv")
            wkv_b = persist.tile([P, CT, 2 * INNER], BF16)
            wkv_dv = wkv_d[:].rearrange("(a p) m -> p a m", p=P)

            # HAM warmup scratch: PE work paced by the arriving input chunks
            # keeps the clock gate at 8/8 through the projection phase.
            warm = psO.tile([D + 1, N], F32, tag="ov", name="warm")

            def warm_mms(src, n):
                for r in range(n):
                    nc.tensor.matmul(
                        warm[0:D, 0:512],
                        lhsT=src[:, (r % 4) * D:(r % 4) * D + D],
                        rhs=src[:, 0:512],
                        start=True, stop=True,
                        skip_group_check=True,
                    )

            for a in range(CT):
                nc.sync.dma_start(out=x_f[:, a, :], in_=x_dv[:, a, :])
                nc.vector.tensor_copy(out=x_b[:, a, :], in_=x_f[:, a, :])
                nc.gpsimd.dma_start(out=wq_f[:, a, :], in_=wq_dv[:, a, :])
                nc.scalar.copy(out=wq_b[:, a, :], in_=wq_f[:, a, :])
                # dense burst on the first chunk trips the HAM SHORT window;
                # a couple of matmuls per later chunk hold it.
                warm_mms(x_b[:, a, :], 12 if a == 0 else 3)
            for a in range(CT):
                nc.sync.dma_start(out=wkv_f[:, a, :], in_=wkv_dv[:, a, :])
                nc.scalar.copy(out=wkv_b[:, a, :], in_=wkv_f[:, a, :])
                warm_mms(wkv_b[:, a, 0:INNER], 3)

            wo_f = stage.tile([P, MT, C], F32, tag="st_q")
            nc.gpsimd.dma_start(
                out=wo_f, in_=wo_d[:].rearrange("(a p) m -> p a m", p=P))
            wo_b = persist.tile([P, MT, C], BF16)
            nc.vector.tensor_copy(out=wo_b, in_=wo_f)

            bo_bc = persist.tile([P, C], F32)
            bo_ap = bo_d[:]
            nc.gpsimd.dma_start(
                out=bo_bc,
                in_=bass.AP(tensor=bo_ap.tensor, offset=bo_ap.offset,
                            ap=[[0, P], [1, C]]),
            )

            zb = persist.tile([P, 1], F32)
            nc.vector.memset(zb, 0.0)

            # ---------------- projections ----------------
            qT = persist.tile([P, MT, N], BF16)
            kT = persist.tile([P, MT, N], BF16)
            v_ext = persist.tile([P, JT, HEADS, D + 1], BF16)
            nc.vector.memset(v_ext[:, :, :, D], 1.0)

            def kq_proj(mt):
                for dst, w_b in ((kT, wkv_b), (qT, wq_b)):
                    for ib in range(NB):
                        ps = (psA if ib == 0 else psB).tile(
                            [P, 512], F32, tag="sim", name="pj")
                        for a in range(CT):
                            nc.tensor.matmul(
                                ps,
                                lhsT=w_b[:, a, mt * P:(mt + 1) * P],
                                rhs=x_b[:, a, ib * 512:(ib + 1) * 512],
                                start=(a == 0),
                                stop=(a == CT - 1),
                            )
                        nc.vector.tensor_copy(
                            out=dst[:, mt, ib * 512:(ib + 1) * 512], in_=ps)

            def v_proj():
                # V in normal layout (token j on partitions), per head with an
                # extra ones column: v_ext[:, jt, h, 0:64] = V, [..., 64] = 1
                for jt in range(JT):
                    ps = (psA if jt % 2 == 0 else psB).tile(
                        [P, 512], F32, tag="sim", name="pv")
                    for a in range(CT):
                        nc.tensor.matmul(
                            ps,
                            lhsT=x_b[:, a, jt * P:(jt + 1) * P],
                            rhs=wkv_b[:, a, INNER:2 * INNER],
                            start=(a == 0),
                            stop=(a == CT - 1),
                        )
                    nc.vector.tensor_copy(
                        out=v_ext[:, jt, :, 0:D],
                        in_=ps.rearrange("p (h d) -> p h d", h=HEADS),
                    )

            for mt in range(MT):
                kq_proj(mt)
            v_proj()

            # ---------------- attention (pair-pipelined) ----------------
            oTs = []
            for m in range(PAIRS):
                oT_m = persist.tile([P, N], BF16, tag=f"oT{m}")
                oTs.append(oT_m)

            et_tiles = {}
            ov_tiles = {}

            def emit_sim(m, jt):
                """Row-tiled query-halved sims. Four [128 keys, 512 q] tiles:
                (h0,q0)+(h1,q1) feed ScalarE exp, (h1,q0)+(h0,q1) feed the
                VectorE fast exp. Tiles of a pair use disjoint PE row groups
                so they run concurrently."""
                et = et_tiles[m]
                ea = []
                for pool, pairs_ in (
                    (psA, ((0, 0), (1, 1))),   # -> ScalarE
                    (psB, ((1, 0), (0, 1))),   # -> VectorE
                ):
                    for hh, qh in pairs_:
                        st = pool.tile([P, 512], F32, tag="sim",
                                       name=f"st{hh}{qh}")
                        hp = hh * D
                        nc.tensor.matmul(
                            st,
                            lhsT=kT[hp:hp + D, m, jt * P:(jt + 1) * P],
                            rhs=qT[hp:hp + D, m, qh * 512:(qh + 1) * 512],
                            start=True, stop=True,
                        )
                        ea.append((hh, qh, st))
                for hh, qh, st in ea[:2]:
                    nc.scalar.activation(
                        out=et[:, hh, jt, qh * 512:(qh + 1) * 512],
                        in_=st, func=EXP, bias=zb, scale=SCALE)
                for hh, qh, st in ea[2:]:
                    nc.vector.tensor_scalar(
                        et[:, hh, jt, qh * 512:(qh + 1) * 512].bitcast(I16),
                        st,
                        FEXP_A,
                        FEXP_B,
                        MULT,
                        ADD,
                    )

            def emit_av(m, jt):
                """attn@v for key tile jt of pair m (one F=1024 matmul per
                head; output [65, 1024] spans 2 PSUM banks)."""
                et = et_tiles[m]
                for hh in range(2):
                    ov = ov_tiles[(m, hh)]
                    if AV_F == 1024:
                        nc.tensor.matmul(
                            ov,
                            lhsT=v_ext[:, jt, 2 * m + hh, :],
                            rhs=et[:, hh, jt, :],
                            start=(jt == 0),
                            stop=(jt == JT - 1),
                        )
                    else:
                        for ib in range(NB):
                            sl = slice(ib * 512, (ib + 1) * 512)
                            nc.tensor.matmul(
                                ov[:, sl],
                                lhsT=v_ext[:, jt, 2 * m + hh, :],
                                rhs=et[:, hh, jt, sl],
                                start=(jt == 0),
                                stop=(jt == JT - 1),
                            )

            def emit_drain(m, hh):
                """ScalarE drains ov (o rows + denom row) to bf16 SBUF and
                kicks off the denominator reciprocal bounce."""
                ov = ov_tiles.pop((m, hh))
                ov_sb = small.tile([D + 1, N], BF16, tag="ovsb")
                nc.scalar.activation(out=ov_sb, in_=ov, func=COPY)
                # denom row -> DRAM -> [128, 8] scatter -> reciprocal
                sd = dramp.tile([N], BF16, tag="sd")
                nc.sync.dma_start(out=sd, in_=ov_sb[D:D + 1, :])
                st2 = small.tile([P, NT], BF16, tag="st2")
                nc.sync.dma_start(
                    out=st2, in_=sd.rearrange("(p k) -> p k", k=NT))
                rst2 = small.tile([P, NT], BF16, tag="rst2")
                with nc.allow_low_precision(
                        reason="softmax denom reciprocal in bf16; "
                        "~0.4% rel err, within the 2e-2 budget"):
                    nc.vector.reciprocal(rst2, st2)
                rsd = dramp.tile([N], BF16, tag="rsd")
                nc.sync.dma_start(
                    out=rsd.rearrange("(p k) -> p k", k=NT), in_=rst2)
                rep = small.tile([D, N], BF16, tag="rep")
                rsd_ap = rsd[:]
                nc.sync.dma_start(
                    out=rep,
                    in_=bass.AP(tensor=rsd_ap.tensor,
                                offset=rsd_ap.offset,
                                ap=[[0, D], [1, N]]),
                )
                return ov_sb, rep

            def emit_norm(m, hh, ov_sb, rep):
                hp = hh * D
                nc.vector.tensor_mul(
                    oTs[m][hp:hp + D, :], ov_sb[0:D, :], rep)

            # deferred-work schedule per slot (m, jt):
            #   jt>=2  : attn@v (m, jt-2)
            #   jt=0,1 : attn@v (m-1, 6) / (m-1, 7)
            #   jt=1,2 : drains of pair m-1 (hh=0,1)
            #   jt=3,4 : norms of pair m-1
            pending_drain = {}
            for m in range(PAIRS):
                et_tiles[m] = etp.tile([P, 2, JT, N], BF16, tag="et",
                                       name=f"et{m}")
                for hh in range(2):
                    ov_tiles[(m, hh)] = psO.tile([D + 1, N], F32, tag="ov",
                                                 name=f"ov{m}_{hh}")
                for jt in range(JT):
                    emit_sim(m, jt)
                    if jt >= 2:
                        emit_av(m, jt - 2)
                    elif m > 0:
                        emit_av(m - 1, JT - 2 + jt)
                    if m > 0:
                        if jt in (1, 2):
                            hh = jt - 1
                            pending_drain[hh] = emit_drain(m - 1, hh)
                        elif jt in (3, 4):
                            hh = jt - 3
                            emit_norm(m - 1, hh, *pending_drain.pop(hh))
            # tail: last pair's remaining chunks + epilogue
            m = PAIRS - 1
            emit_av(m, JT - 2)
            emit_av(m, JT - 1)
            d0 = emit_drain(m, 0)
            d1 = emit_drain(m, 1)
            # keep the PE clock warm across the epilogue-latency bubble
            warm2 = psO.tile([D + 1, N], F32, tag="ov", name="warm2")
            for r in range(8):
                nc.tensor.matmul(
                    warm2[0:D, 0:512],
                    lhsT=x_b[:, r % 4, 0:D],
                    rhs=x_b[:, r % 4, 0:512],
                    start=True, stop=True,
                    skip_group_check=True,
                )
            emit_norm(m, 0, *d0)
            emit_norm(m, 1, *d1)

            # ---------------- output projection ----------------
            for it in range(NT):
                pf = (psA if it % 2 == 0 else psB).tile(
                    [P, 512], F32, tag="sim", name="op")
                for kk in range(MT):
                    nc.tensor.matmul(
                        pf,
                        lhsT=oTs[kk][:, it * P:(it + 1) * P],
                        rhs=wo_b[:, kk, :],
                        start=(kk == 0),
                        stop=(kk == MT - 1),
                    )
                fin = small.tile([P, C], F32, tag="fin")
                nc.vector.tensor_add(fin, pf, bo_bc)
                nc.sync.dma_start(out=out_d[it * P:(it + 1) * P, :], in_=fin)

    return nc


def kernel(x, Wq, Wkv, Wo, bo):
    from concourse.bass_utils import run_bass_kernel_spmd

    nc = build_nc()
    nc.compile()
    x = np.asarray(x)
    xs = np.ascontiguousarray(x.reshape(B, C, N)).astype(np.float32, copy=False)
    in_maps = [
        {
            "x": xs[b],
            "Wq": np.asarray(Wq, dtype=np.float32),
            "Wkv": np.asarray(Wkv, dtype=np.float32),
            "Wo": np.asarray(Wo, dtype=np.float32),
            "bo": np.asarray(bo, dtype=np.float32),
        }
        for b in range(B)
    ]
    res = run_bass_kernel_spmd(nc, in_maps, list(range(B)))
    return np.stack([res.results[b]["out"] for b in range(B)], axis=0)


# revision 7
# speedup vs baseline: 1.0448x; 1.0191x over previous
"""Self-contained Trainium2 Bass kernel for nn_Attention (8-head self-attention).

Reference computation (per batch element b):
    xt = x[b].reshape(C, N).T            # (N, C),  N = H*W = 1024
    q  = xt @ Wq                         # (N, 512)
    k, v = split(xt @ Wkv)               # (N, 512) each
    per head h (d=64): sim = q_h k_h^T / 8 ; P = softmax(sim) ; o_h = P v_h
    out[b] = concat_h(o_h) @ Wo + bo     # (N, C)

Sharding: pure data parallel -- core b computes batch element b (8 cores, 8
batch elements, no collectives).

Layout strategy (keeps every matmul contraction dim on partitions, zero
on-chip transposes):
  - x[b] is used as (C, N): already the transpose of xt.
  - qT, kT are produced in (inner, N) layout; V in (N, inner) layout with an
    extra ones-column per head so the P@V matmul also emits the softmax
    denominators (M = 64+1 = 65).
  - Heads are processed in PAIRS (2m, 2m+1). Head 2m lives on partitions
    0:64 of qT/kT[:, m, :], head 2m+1 on 64:128, so their K=64 sim matmuls
    occupy disjoint PE row groups and execute CONCURRENTLY (row tiling).
  - Sim tiles are QUERY-HALVED: [128 keys, 512 queries] = one PSUM bank,
    so each exp consumer gets a double-buffered chain (psA/psB bufs=2) and
    the sim -> exp -> sim PSUM-reuse chain never serializes the slot.
  - exp is split across two engines: ScalarE runs the real activation on
    two of the four (head, qhalf) tiles per key-tile step, VectorE runs a
    Schraudolph-style fast exp (affine in f32, convert to int16 = the bf16
    bit pattern of exp, bitcast back to bf16) on the other two. Fast-exp
    max rel err ~3% pre-softmax; the systematic part cancels in the
    normalization; end-to-end contribution ~3e-3.
  - attn@v uses F=1024 bf16 moving operands (one matmul per (head, key
    tile), output spans 2 PSUM banks).
  - Pipelining: attn@v for key tile jt runs at slot jt+2; the previous
    pair's epilogue (ScalarE PSUM drain, denominator reciprocal via a
    DRAM scatter bounce, bf16 normalize multiply on VectorE) occupies the
    next pair's early slots.
  - HAM warmup: dummy matmuls paced by the input-load casts keep the PE
    activity monitor from idling the clock to 1.2 GHz before/through the
    compute phases.
  - PSUM budget: 4 sim banks + two [65, N] attn@v accumulators (2 banks
    each, double buffered across pairs) = 8 banks.
"""

import numpy as np

import concourse.bass as bass
import concourse.mybir as mybir
import concourse.tile as tile
from concourse import bacc

B, C, N = 8, 512, 1024
HEADS, D = 8, 64
INNER = HEADS * D  # 512
SCALE = D ** -0.5
P = 128
CT = C // P       # 4  k-tiles over C
MT = INNER // P   # 4  partition-tiles over inner
JT = N // P       # 8  key tiles
NT = N // P       # 8  output row tiles
NB = N // 512     # 2  free-dim blocks of 512 over N
PAIRS = HEADS // 2

F32 = mybir.dt.float32
BF16 = mybir.dt.bfloat16
I16 = mybir.dt.int16
EXP = mybir.ActivationFunctionType.Exp
COPY = mybir.ActivationFunctionType.Copy
MULT = mybir.AluOpType.mult
ADD = mybir.AluOpType.add

# fast-exp constants: bf16 bits of exp(s * SCALE) ~= int16(FEXP_A * s + FEXP_B)
FEXP_A = 128.0 * 1.4426950408889634 * SCALE
FEXP_B = 16250.65

AV_F = 512  # attn@v free size (matmul output cannot cross a PSUM bank)


def build_nc(debug=False):
    nc = bacc.Bacc(
        "TRN2", target_bir_lowering=False, debug=debug, num_devices=B
    )
    x_d = nc.dram_tensor("x", [C, N], F32, kind="ExternalInput")
    wq_d = nc.dram_tensor("Wq", [C, INNER], F32, kind="ExternalInput")
    wkv_d = nc.dram_tensor("Wkv", [C, 2 * INNER], F32, kind="ExternalInput")
    wo_d = nc.dram_tensor("Wo", [INNER, C], F32, kind="ExternalInput")
    bo_d = nc.dram_tensor("bo", [C], F32, kind="ExternalInput")
    out_d = nc.dram_tensor("out", [N, C], F32, kind="ExternalOutput")

    with tile.TileContext(nc) as tc:
        with (
            tc.tile_pool(name="persist", bufs=1) as persist,
            tc.tile_pool(name="stage", bufs=1) as stage,
            tc.tile_pool(name="etp", bufs=2) as etp,
            tc.tile_pool(name="small", bufs=4) as small,
            tc.tile_pool(name="dramp", bufs=2, space="DRAM") as dramp,
            tc.tile_pool(name="psA", bufs=2, space="PSUM") as psA,
            tc.tile_pool(name="psB", bufs=2, space="PSUM") as psB,
            tc.tile_pool(name="psO", bufs=2, space="PSUM") as psO,
        ):
            # ---------------- load + cast inputs ----------------
            # x + wkv on the sync DMA queue, wq + wo + bo on the gpsimd
            # queue so the transfers overlap.
            x_f = stage.tile([P, CT, N], F32, tag="st_x")
            x_b = persist.tile([P, CT, N], BF16)
            x_dv = x_d[:].rearrange("(a p) n -> p a n", p=P)
            wq_f = stage.tile([P, CT, INNER], F32, tag="st_q")
            wq_b = persist.tile([P, CT, INNER], BF16)
            wq_dv = wq_d[:].rearrange("(a p) m -> p a m", p=P)
            wkv_f = stage.tile([P, CT, 2 * INNER], F32, tag="st_kv")
            wkv_b = persist.tile([P, CT, 2 * INNER], BF16)
            wkv_dv = wkv_d[:].rearrange("(a p) m -> p a m", p=P)

            # HAM warmup scratch: PE work paced by the arriving input chunks
            # keeps the clock gate at 8/8 through the projection phase.
            warm = psO.tile([D + 1, N], F32, tag="ov", name="warm")

            def warm_mms(src, n):
                for r in range(n):
                    nc.tensor.matmul(
                        warm[0:D, 0:512],
                        lhsT=src[:, (r % 4) * D:(r % 4) * D + D],
                        rhs=src[:, 0:512],
                        start=True, stop=True,
                        skip_group_check=True,
                    )

            for a in range(CT):
                nc.sync.dma_start(out=x_f[:, a, :], in_=x_dv[:, a, :])
                nc.gpsimd.tensor_copy(out=x_b[:, a, :], in_=x_f[:, a, :])
                nc.gpsimd.dma_start(out=wq_f[:, a, :], in_=wq_dv[:, a, :])
                nc.scalar.copy(out=wq_b[:, a, :], in_=wq_f[:, a, :])
                # dense burst on the first chunk trips the HAM SHORT window;
                # a couple of matmuls per later chunk hold it.
                warm_mms(x_b[:, a, :], 12 if a == 0 else 3)
            for a in range(CT):
                nc.sync.dma_start(out=wkv_f[:, a, :], in_=wkv_dv[:, a, :])
                nc.scalar.copy(out=wkv_b[:, a, :], in_=wkv_f[:, a, :])
                warm_mms(wkv_b[:, a, 0:INNER], 3)

            wo_f = stage.tile([P, MT, C], F32, tag="st_q")
            nc.gpsimd.dma_start(
                out=wo_f, in_=wo_d[:].rearrange("(a p) m -> p a m", p=P))
            wo_b = persist.tile([P, MT, C], BF16)
            nc.gpsimd.tensor_copy(out=wo_b, in_=wo_f)

            bo_bc = persist.tile([P, C], F32)
            bo_ap = bo_d[:]
            nc.gpsimd.dma_start(
                out=bo_bc,
                in_=bass.AP(tensor=bo_ap.tensor, offset=bo_ap.offset,
                            ap=[[0, P], [1, C]]),
            )

            zb = persist.tile([P, 1], F32)
            nc.vector.memset(zb, 0.0)

            # ---------------- projections ----------------
            qT = persist.tile([P, MT, N], BF16)
            kT = persist.tile([P, MT, N], BF16)
            v_ext = persist.tile([P, JT, HEADS, D + 1], BF16)
            nc.vector.memset(v_ext[:, :, :, D], 1.0)

            def kq_proj(mt):
                for dst, w_b in ((kT, wkv_b), (qT, wq_b)):
                    for ib in range(NB):
                        ps = (psA if ib == 0 else psB).tile(
                            [P, 512], F32, tag="sim", name="pj")
                        for a in range(CT):
                            nc.tensor.matmul(
                                ps,
                                lhsT=w_b[:, a, mt * P:(mt + 1) * P],
                                rhs=x_b[:, a, ib * 512:(ib + 1) * 512],
                                start=(a == 0),
                                stop=(a == CT - 1),
                            )
                        nc.vector.tensor_copy(
                            out=dst[:, mt, ib * 512:(ib + 1) * 512], in_=ps)

            def v_proj():
                # V in normal layout (token j on partitions), per head with an
                # extra ones column: v_ext[:, jt, h, 0:64] = V, [..., 64] = 1
                for jt in range(JT):
                    ps = (psA if jt % 2 == 0 else psB).tile(
                        [P, 512], F32, tag="sim", name="pv")
                    for a in range(CT):
                        nc.tensor.matmul(
                            ps,
                            lhsT=x_b[:, a, jt * P:(jt + 1) * P],
                            rhs=wkv_b[:, a, INNER:2 * INNER],
                            start=(a == 0),
                            stop=(a == CT - 1),
                        )
                    nc.vector.tensor_copy(
                        out=v_ext[:, jt, :, 0:D],
                        in_=ps.rearrange("p (h d) -> p h d", h=HEADS),
                    )

            for mt in range(MT):
                kq_proj(mt)
            v_proj()

            # ---------------- attention (pair-pipelined) ----------------
            oTs = []
            for m in range(PAIRS):
                oT_m = persist.tile([P, N], BF16, tag=f"oT{m}")
                oTs.append(oT_m)

            et_tiles = {}
            ov_tiles = {}

            def emit_sim(m, jt):
                """Row-tiled query-halved sims. Four [128 keys, 512 q] tiles:
                (h0,q0)+(h1,q1) feed ScalarE exp, (h1,q0)+(h0,q1) feed the
                VectorE fast exp. Tiles of a pair use disjoint PE row groups
                so they run concurrently."""
                et = et_tiles[m]
                ea = []
                for pool, pairs_ in (
                    (psA, ((0, 0), (1, 1))),
                    (psB, ((1, 0), (0, 1))),
                ):
                    for hh, qh in pairs_:
                        st = pool.tile([P, 512], F32, tag="sim",
                                       name=f"st{hh}{qh}")
                        hp = hh * D
                        nc.tensor.matmul(
                            st,
                            lhsT=kT[hp:hp + D, m, jt * P:(jt + 1) * P],
                            rhs=qT[hp:hp + D, m, qh * 512:(qh + 1) * 512],
                            start=True, stop=True,
                        )
                        ea.append((hh, qh, st))
                # each pool's two tiles drain on DIFFERENT engines so both
                # buffers release concurrently: ScalarE gets ea[0] (psA) +
                # ea[2] (psB), VectorE gets ea[1] (psA) + ea[3] (psB).
                for hh, qh, st in (ea[0], ea[2]):
                    nc.scalar.activation(
                        out=et[:, hh, jt, qh * 512:(qh + 1) * 512],
                        in_=st, func=EXP, bias=zb, scale=SCALE)
                for hh, qh, st in (ea[1], ea[3]):
                    nc.vector.tensor_scalar(
                        et[:, hh, jt, qh * 512:(qh + 1) * 512].bitcast(I16),
                        st,
                        FEXP_A,
                        FEXP_B,
                        MULT,
                        ADD,
                    )

            def emit_av(m, jt):
                """attn@v for key tile jt of pair m (one F=1024 matmul per
                head; output [65, 1024] spans 2 PSUM banks)."""
                et = et_tiles[m]
                for hh in range(2):
                    ov = ov_tiles[(m, hh)]
                    if AV_F == 1024:
                        nc.tensor.matmul(
                            ov,
                            lhsT=v_ext[:, jt, 2 * m + hh, :],
                            rhs=et[:, hh, jt, :],
                            start=(jt == 0),
                            stop=(jt == JT - 1),
                        )
                    else:
                        for ib in range(NB):
                            sl = slice(ib * 512, (ib + 1) * 512)
                            nc.tensor.matmul(
                                ov[:, sl],
                                lhsT=v_ext[:, jt, 2 * m + hh, :],
                                rhs=et[:, hh, jt, sl],
                                start=(jt == 0),
                                stop=(jt == JT - 1),
                            )

            def emit_drain(m, hh):
                """ScalarE drains ov (o rows + denom row) to bf16 SBUF and
                kicks off the denominator reciprocal bounce."""
                ov = ov_tiles.pop((m, hh))
                ov_sb = small.tile([D + 1, N], BF16, tag="ovsb")
                nc.scalar.activation(out=ov_sb, in_=ov, func=COPY)
                # denom row -> DRAM -> [128, 8] scatter -> reciprocal
                sd = dramp.tile([N], BF16, tag="sd")
                nc.sync.dma_start(out=sd, in_=ov_sb[D:D + 1, :])
                st2 = small.tile([P, NT], BF16, tag="st2")
                nc.sync.dma_start(
                    out=st2, in_=sd.rearrange("(p k) -> p k", k=NT))
                rst2 = small.tile([P, NT], BF16, tag="rst2")
                with nc.allow_low_precision(
                        reason="softmax denom reciprocal in bf16; "
                        "~0.4% rel err, within the 2e-2 budget"):
                    nc.vector.reciprocal(rst2, st2)
                rsd = dramp.tile([N], BF16, tag="rsd")
                nc.sync.dma_start(
                    out=rsd.rearrange("(p k) -> p k", k=NT), in_=rst2)
                rep = small.tile([D, N], BF16, tag="rep")
                rsd_ap = rsd[:]
                nc.sync.dma_start(
                    out=rep,
                    in_=bass.AP(tensor=rsd_ap.tensor,
                                offset=rsd_ap.offset,
                                ap=[[0, D], [1, N]]),
                )
                return ov_sb, rep

            def emit_norm(m, hh, ov_sb, rep):
                hp = hh * D
                nc.vector.tensor_mul(
                    oTs[m][hp:hp + D, :], ov_sb[0:D, :], rep)

            # deferred-work schedule per slot (m, jt):
            #   jt>=2  : attn@v (m, jt-2)
            #   jt=0,1 : attn@v (m-1, 6) / (m-1, 7)
            #   jt=1,2 : drains of pair m-1 (hh=0,1)
            #   jt=3,4 : norms of pair m-1
            pending_drain = {}
            for m in range(PAIRS):
                et_tiles[m] = etp.tile([P, 2, JT, N], BF16, tag="et",
                                       name=f"et{m}")
                for hh in range(2):
                    ov_tiles[(m, hh)] = psO.tile([D + 1, N], F32, tag="ov",
                                                 name=f"ov{m}_{hh}")
                for jt in range(JT):
                    emit_sim(m, jt)
                    if jt >= 2:
                        emit_av(m, jt - 2)
                    elif m > 0:
                        emit_av(m - 1, JT - 2 + jt)
                    if m > 0:
                        if jt in (1, 2):
                            hh = jt - 1
                            pending_drain[hh] = emit_drain(m - 1, hh)
                        elif jt in (3, 4):
                            hh = jt - 3
                            emit_norm(m - 1, hh, *pending_drain.pop(hh))
            # tail: last pair's remaining chunks + epilogue, overlapped
            # with the first output-projection accumulations (kk=0..2 do not
            # need the still-draining pair-3 heads).
            m = PAIRS - 1
            emit_av(m, JT - 2)
            emit_av(m, JT - 1)
            d0 = emit_drain(m, 0)
            d1 = emit_drain(m, 1)
            op_tiles = {}

            def op_head(its):
                for it in its:
                    pf = (psA if it % 2 == 0 else psB).tile(
                        [P, 512], F32, tag="sim", name="op")
                    op_tiles[it] = pf
                    for kk in range(MT - 1):
                        nc.tensor.matmul(
                            pf,
                            lhsT=oTs[kk][:, it * P:(it + 1) * P],
                            rhs=wo_b[:, kk, :],
                            start=(kk == 0),
                            stop=False,
                        )

            def op_tail(its):
                kk = MT - 1
                for it in its:
                    pf = op_tiles.pop(it)
                    nc.tensor.matmul(
                        pf,
                        lhsT=oTs[kk][:, it * P:(it + 1) * P],
                        rhs=wo_b[:, kk, :],
                        start=False,
                        stop=True,
                    )
                    fin = small.tile([P, C], F32, tag="fin")
                    nc.vector.tensor_add(fin, pf, bo_bc)
                    nc.sync.dma_start(
                        out=out_d[it * P:(it + 1) * P, :], in_=fin)

            op_head(range(0, 4))
            emit_norm(m, 0, *d0)
            emit_norm(m, 1, *d1)
            op_tail(range(0, 4))
            op_head(range(4, NT))
            op_tail(range(4, NT))

    return nc


def kernel(x, Wq, Wkv, Wo, bo):
    from concourse.bass_utils import run_bass_kernel_spmd

    nc = build_nc()
    nc.compile()
    x = np.asarray(x)
    xs = np.ascontiguousarray(x.reshape(B, C, N)).astype(np.float32, copy=False)
    in_maps = [
        {
            "x": xs[b],
            "Wq": np.asarray(Wq, dtype=np.float32),
            "Wkv": np.asarray(Wkv, dtype=np.float32),
            "Wo": np.asarray(Wo, dtype=np.float32),
            "bo": np.asarray(bo, dtype=np.float32),
        }
        for b in range(B)
    ]
    res = run_bass_kernel_spmd(nc, in_maps, list(range(B)))
    return np.stack([res.results[b]["out"] for b in range(B)], axis=0)


# revision 8
# speedup vs baseline: 1.0826x; 1.0361x over previous
"""Self-contained Trainium2 Bass kernel for nn_Attention (8-head self-attention).

Reference computation (per batch element b):
    xt = x[b].reshape(C, N).T            # (N, C),  N = H*W = 1024
    q  = xt @ Wq                         # (N, 512)
    k, v = split(xt @ Wkv)               # (N, 512) each
    per head h (d=64): sim = q_h k_h^T / 8 ; P = softmax(sim) ; o_h = P v_h
    out[b] = concat_h(o_h) @ Wo + bo     # (N, C)

Sharding: pure data parallel -- core b computes batch element b (8 cores, 8
batch elements, no collectives).

Layout strategy (keeps every matmul contraction dim on partitions, zero
on-chip transposes):
  - x[b] is used as (C, N): already the transpose of xt.
  - qT, kT are produced in (inner, N) layout; V in (N, inner) layout with an
    extra ones-column per head so the P@V matmul also emits the softmax
    denominators (M = 64+1 = 65).
  - Heads are processed in PAIRS (2m, 2m+1). Head 2m lives on partitions
    0:64 of qT/kT[:, m, :], head 2m+1 on 64:128, so their K=64 sim matmuls
    occupy disjoint PE row groups and execute CONCURRENTLY (row tiling).
  - Sim tiles are QUERY-HALVED: [128 keys, 512 queries] = one PSUM bank,
    so each exp consumer gets a double-buffered chain (psA/psB bufs=2) and
    the sim -> exp -> sim PSUM-reuse chain never serializes the slot.
  - exp is split across two engines: ScalarE runs the real activation on
    two of the four (head, qhalf) tiles per key-tile step, VectorE runs a
    Schraudolph-style fast exp (affine in f32, convert to int16 = the bf16
    bit pattern of exp, bitcast back to bf16) on the other two. Fast-exp
    max rel err ~3% pre-softmax; the systematic part cancels in the
    normalization; end-to-end contribution ~3e-3.
  - attn@v uses F=1024 bf16 moving operands (one matmul per (head, key
    tile), output spans 2 PSUM banks).
  - Pipelining: attn@v for key tile jt runs at slot jt+2; the previous
    pair's epilogue (ScalarE PSUM drain, denominator reciprocal via a
    DRAM scatter bounce, bf16 normalize multiply on VectorE) occupies the
    next pair's early slots.
  - HAM warmup: dummy matmuls paced by the input-load casts keep the PE
    activity monitor from idling the clock to 1.2 GHz before/through the
    compute phases.
  - PSUM budget: 4 sim banks + two [65, N] attn@v accumulators (2 banks
    each, double buffered across pairs) = 8 banks.
"""

import numpy as np

import concourse.bass as bass
import concourse.mybir as mybir
import concourse.tile as tile
from concourse import bacc

B, C, N = 8, 512, 1024
HEADS, D = 8, 64
INNER = HEADS * D  # 512
SCALE = D ** -0.5
P = 128
CT = C // P       # 4  k-tiles over C
MT = INNER // P   # 4  partition-tiles over inner
JT = N // P       # 8  key tiles
NT = N // P       # 8  output row tiles
NB = N // 512     # 2  free-dim blocks of 512 over N
PAIRS = HEADS // 2

F32 = mybir.dt.float32
BF16 = mybir.dt.bfloat16
I16 = mybir.dt.int16
EXP = mybir.ActivationFunctionType.Exp
COPY = mybir.ActivationFunctionType.Copy
MULT = mybir.AluOpType.mult
ADD = mybir.AluOpType.add

# fast-exp constants: bf16 bits of exp(s * SCALE) ~= int16(FEXP_A * s + FEXP_B)
FEXP_A = 128.0 * 1.4426950408889634 * SCALE
FEXP_B = 16250.65

AV_F = 512  # attn@v free size (matmul output cannot cross a PSUM bank)


def build_nc(debug=False):
    nc = bacc.Bacc(
        "TRN2", target_bir_lowering=False, debug=debug, num_devices=B
    )
    x_d = nc.dram_tensor("x", [C, N], F32, kind="ExternalInput")
    wq_d = nc.dram_tensor("Wq", [C, INNER], F32, kind="ExternalInput")
    wkv_d = nc.dram_tensor("Wkv", [C, 2 * INNER], F32, kind="ExternalInput")
    wo_d = nc.dram_tensor("Wo", [INNER, C], F32, kind="ExternalInput")
    bo_d = nc.dram_tensor("bo", [C], F32, kind="ExternalInput")
    out_d = nc.dram_tensor("out", [N, C], F32, kind="ExternalOutput")

    with tile.TileContext(nc) as tc:
        with (
            tc.tile_pool(name="persist", bufs=1) as persist,
            tc.tile_pool(name="stage", bufs=1) as stage,
            tc.tile_pool(name="etp", bufs=2) as etp,
            tc.tile_pool(name="small", bufs=4) as small,
            tc.tile_pool(name="dramp", bufs=2, space="DRAM") as dramp,
            tc.tile_pool(name="psA", bufs=2, space="PSUM") as psA,
            tc.tile_pool(name="psB", bufs=2, space="PSUM") as psB,
            tc.tile_pool(name="psO", bufs=2, space="PSUM") as psO,
        ):
            # ---------------- load + cast inputs ----------------
            # x + wkv on the sync DMA queue, wq + wo + bo on the gpsimd
            # queue so the transfers overlap.
            x_f = stage.tile([P, CT, N], F32, tag="st_x")
            x_b = persist.tile([P, CT, N], BF16)
            x_dv = x_d[:].rearrange("(a p) n -> p a n", p=P)
            wq_f = stage.tile([P, CT, INNER], F32, tag="st_q")
            wq_b = persist.tile([P, CT, INNER], BF16)
            wq_dv = wq_d[:].rearrange("(a p) m -> p a m", p=P)
            wkv_f = stage.tile([P, CT, 2 * INNER], F32, tag="st_kv")
            wkv_b = persist.tile([P, CT, 2 * INNER], BF16)
            wkv_dv = wkv_d[:].rearrange("(a p) m -> p a m", p=P)

            # HAM warmup scratch: PE work paced by the arriving input chunks
            # keeps the clock gate at 8/8 through the projection phase.
            warm = psO.tile([D + 1, N], F32, tag="ov", name="warm")

            def warm_mms(src, n):
                for r in range(n):
                    nc.tensor.matmul(
                        warm[0:D, 0:512],
                        lhsT=src[:, (r % 4) * D:(r % 4) * D + D],
                        rhs=src[:, 0:512],
                        start=True, stop=True,
                        skip_group_check=True,
                    )

            wo_f = stage.tile([P, MT, C], F32, tag="st_wo")
            bo_bc = persist.tile([P, C], F32)
            bo_ap = bo_d[:]
            # queue all input DMAs first: x + wkv on sync, wq + wo + bo on
            # gpsimd, so the two queues stream concurrently.
            for a in range(CT):
                nc.sync.dma_start(out=x_f[:, a, :], in_=x_dv[:, a, :])
                nc.gpsimd.dma_start(out=wq_f[:, a, :], in_=wq_dv[:, a, :])
            for a in range(CT):
                nc.sync.dma_start(out=wkv_f[:, a, :], in_=wkv_dv[:, a, :])
            nc.gpsimd.dma_start(
                out=wo_f, in_=wo_d[:].rearrange("(a p) m -> p a m", p=P))
            nc.gpsimd.dma_start(
                out=bo_bc,
                in_=bass.AP(tensor=bo_ap.tensor, offset=bo_ap.offset,
                            ap=[[0, P], [1, C]]),
            )
            # casts as chunks land; warmup matmuls paced by the x casts
            for a in range(CT):
                nc.vector.tensor_copy(out=x_b[:, a, :], in_=x_f[:, a, :])
                nc.scalar.copy(out=wq_b[:, a, :], in_=wq_f[:, a, :])
                warm_mms(x_b[:, a, :], 12 if a == 0 else 3)
            for a in range(CT):
                nc.scalar.copy(out=wkv_b[:, a, :], in_=wkv_f[:, a, :])
                warm_mms(wkv_b[:, a, 0:INNER], 3)
            wo_b = persist.tile([P, MT, C], BF16)
            nc.gpsimd.tensor_copy(out=wo_b, in_=wo_f)

            zb = persist.tile([P, 1], F32)
            nc.vector.memset(zb, 0.0)

            # ---------------- projections ----------------
            qT = persist.tile([P, MT, N], BF16)
            kT = persist.tile([P, MT, N], BF16)
            v_ext = persist.tile([P, JT, HEADS, D + 1], BF16)
            nc.vector.memset(v_ext[:, :, :, D], 1.0)

            def kq_proj(mt, which=None):
                for dst, w_b in ((kT, wkv_b), (qT, wq_b)):
                    if which is not None and dst is not which:
                        continue
                    for ib in range(NB):
                        ps = (psA if ib == 0 else psB).tile(
                            [P, 512], F32, tag="sim", name="pj")
                        for a in range(CT):
                            nc.tensor.matmul(
                                ps,
                                lhsT=w_b[:, a, mt * P:(mt + 1) * P],
                                rhs=x_b[:, a, ib * 512:(ib + 1) * 512],
                                start=(a == 0),
                                stop=(a == CT - 1),
                            )
                        nc.vector.tensor_copy(
                            out=dst[:, mt, ib * 512:(ib + 1) * 512], in_=ps)

            def v_proj():
                # V in normal layout (token j on partitions), per head with an
                # extra ones column: v_ext[:, jt, h, 0:64] = V, [..., 64] = 1
                for jt in range(JT):
                    ps = (psA if jt % 2 == 0 else psB).tile(
                        [P, 512], F32, tag="sim", name="pv")
                    for a in range(CT):
                        nc.tensor.matmul(
                            ps,
                            lhsT=x_b[:, a, jt * P:(jt + 1) * P],
                            rhs=wkv_b[:, a, INNER:2 * INNER],
                            start=(a == 0),
                            stop=(a == CT - 1),
                        )
                    nc.vector.tensor_copy(
                        out=v_ext[:, jt, :, 0:D],
                        in_=ps.rearrange("p (h d) -> p h d", h=HEADS),
                    )

            for mt in range(MT):
                kq_proj(mt, which=qT)
            for mt in range(MT):
                kq_proj(mt, which=kT)
            v_proj()

            # ---------------- attention (pair-pipelined) ----------------
            oTs = []
            for m in range(PAIRS):
                oT_m = persist.tile([P, N], BF16, tag=f"oT{m}")
                oTs.append(oT_m)

            et_tiles = {}
            ov_tiles = {}

            def emit_sim(m, jt):
                """Row-tiled query-halved sims. Four [128 keys, 512 q] tiles:
                (h0,q0)+(h1,q1) feed ScalarE exp, (h1,q0)+(h0,q1) feed the
                VectorE fast exp. Tiles of a pair use disjoint PE row groups
                so they run concurrently."""
                et = et_tiles[m]
                ea = []
                for pool, pairs_ in (
                    (psA, ((0, 0), (1, 1))),
                    (psB, ((1, 0), (0, 1))),
                ):
                    for hh, qh in pairs_:
                        st = pool.tile([P, 512], F32, tag="sim",
                                       name=f"st{hh}{qh}")
                        hp = hh * D
                        nc.tensor.matmul(
                            st,
                            lhsT=kT[hp:hp + D, m, jt * P:(jt + 1) * P],
                            rhs=qT[hp:hp + D, m, qh * 512:(qh + 1) * 512],
                            start=True, stop=True,
                        )
                        ea.append((hh, qh, st))
                # each pool's two tiles drain on DIFFERENT engines so both
                # buffers release concurrently: ScalarE gets ea[0] (psA) +
                # ea[2] (psB), VectorE gets ea[1] (psA) + ea[3] (psB).
                for hh, qh, st in (ea[0], ea[2]):
                    nc.scalar.activation(
                        out=et[:, hh, jt, qh * 512:(qh + 1) * 512],
                        in_=st, func=EXP, bias=zb, scale=SCALE)
                for hh, qh, st in (ea[1], ea[3]):
                    nc.vector.tensor_scalar(
                        et[:, hh, jt, qh * 512:(qh + 1) * 512].bitcast(I16),
                        st,
                        FEXP_A,
                        FEXP_B,
                        MULT,
                        ADD,
                    )

            def emit_av(m, jt):
                """attn@v for key tile jt of pair m (one F=1024 matmul per
                head; output [65, 1024] spans 2 PSUM banks)."""
                et = et_tiles[m]
                for hh in range(2):
                    ov = ov_tiles[(m, hh)]
                    if AV_F == 1024:
                        nc.tensor.matmul(
                            ov,
                            lhsT=v_ext[:, jt, 2 * m + hh, :],
                            rhs=et[:, hh, jt, :],
                            start=(jt == 0),
                            stop=(jt == JT - 1),
                        )
                    else:
                        for ib in range(NB):
                            sl = slice(ib * 512, (ib + 1) * 512)
                            nc.tensor.matmul(
                                ov[:, sl],
                                lhsT=v_ext[:, jt, 2 * m + hh, :],
                                rhs=et[:, hh, jt, sl],
                                start=(jt == 0),
                                stop=(jt == JT - 1),
                            )

            def emit_drain(m, hh):
                """ScalarE drains ov (o rows + denom row) to bf16 SBUF and
                kicks off the denominator reciprocal bounce."""
                ov = ov_tiles.pop((m, hh))
                ov_sb = small.tile([D + 1, N], BF16, tag="ovsb")
                for ib in range(NB):
                    sl = slice(ib * 512, (ib + 1) * 512)
                    nc.scalar.activation(
                        out=ov_sb[:, sl], in_=ov[:, sl], func=COPY)
                # denom row -> DRAM -> [128, 8] scatter -> reciprocal
                sd = dramp.tile([N], BF16, tag="sd")
                nc.sync.dma_start(out=sd, in_=ov_sb[D:D + 1, :])
                st2 = small.tile([P, NT], BF16, tag="st2")
                nc.sync.dma_start(
                    out=st2, in_=sd.rearrange("(p k) -> p k", k=NT))
                rst2 = small.tile([P, NT], BF16, tag="rst2")
                with nc.allow_low_precision(
                        reason="softmax denom reciprocal in bf16; "
                        "~0.4% rel err, within the 2e-2 budget"):
                    nc.vector.reciprocal(rst2, st2)
                rsd = dramp.tile([N], BF16, tag="rsd")
                nc.sync.dma_start(
                    out=rsd.rearrange("(p k) -> p k", k=NT), in_=rst2)
                rep = small.tile([D, N], BF16, tag="rep")
                rsd_ap = rsd[:]
                nc.sync.dma_start(
                    out=rep,
                    in_=bass.AP(tensor=rsd_ap.tensor,
                                offset=rsd_ap.offset,
                                ap=[[0, D], [1, N]]),
                )
                return ov_sb, rep

            def emit_norm(m, hh, ov_sb, rep, eng=None):
                hp = hh * D
                eng = eng or nc.gpsimd
                eng.tensor_mul(
                    oTs[m][hp:hp + D, :], ov_sb[0:D, :], rep)

            # deferred-work schedule per slot (m, jt):
            #   jt>=2  : attn@v (m, jt-2)
            #   jt=0,1 : attn@v (m-1, 6) / (m-1, 7)
            #   jt=1,2 : drains of pair m-1 (hh=0,1)
            #   jt=3,4 : norms of pair m-1
            pending_drain = {}
            for m in range(PAIRS):
                et_tiles[m] = etp.tile([P, 2, JT, N], BF16, tag="et",
                                       name=f"et{m}")
                for hh in range(2):
                    ov_tiles[(m, hh)] = psO.tile([D + 1, N], F32, tag="ov",
                                                 name=f"ov{m}_{hh}")
                for jt in range(JT):
                    emit_sim(m, jt)
                    if jt >= 2:
                        emit_av(m, jt - 2)
                    elif m > 0:
                        emit_av(m - 1, JT - 2 + jt)
                    if m > 0:
                        if jt in (1, 2):
                            hh = jt - 1
                            pending_drain[hh] = emit_drain(m - 1, hh)
                        elif jt in (3, 4):
                            hh = jt - 3
                            emit_norm(m - 1, hh, *pending_drain.pop(hh))
            # tail: last pair's remaining chunks + epilogue, overlapped
            # with the first output-projection accumulations (kk=0..2 do not
            # need the still-draining pair-3 heads).
            m = PAIRS - 1
            emit_av(m, JT - 2)
            emit_av(m, JT - 1)
            d0 = emit_drain(m, 0)
            d1 = emit_drain(m, 1)
            op_tiles = {}

            def op_head(its):
                for it in its:
                    pf = (psA if it % 2 == 0 else psB).tile(
                        [P, 512], F32, tag="sim", name="op")
                    op_tiles[it] = pf
                    for kk in range(MT - 1):
                        nc.tensor.matmul(
                            pf,
                            lhsT=oTs[kk][:, it * P:(it + 1) * P],
                            rhs=wo_b[:, kk, :],
                            start=(kk == 0),
                            stop=False,
                        )

            def op_tail(its):
                kk = MT - 1
                for it in its:
                    pf = op_tiles.pop(it)
                    nc.tensor.matmul(
                        pf,
                        lhsT=oTs[kk][:, it * P:(it + 1) * P],
                        rhs=wo_b[:, kk, :],
                        start=False,
                        stop=True,
                    )
                    fin = small.tile([P, C], F32, tag="fin")
                    nc.vector.tensor_add(fin, pf, bo_bc)
                    nc.sync.dma_start(
                        out=out_d[it * P:(it + 1) * P, :], in_=fin)

            op_head(range(0, 4))
            emit_norm(m, 0, *d0, eng=nc.vector)
            emit_norm(m, 1, *d1, eng=nc.vector)
            op_tail(range(0, 4))
            op_head(range(4, NT))
            op_tail(range(4, NT))

    return nc


def kernel(x, Wq, Wkv, Wo, bo):
    from concourse.bass_utils import run_bass_kernel_spmd

    nc = build_nc()
    nc.compile()
    x = np.asarray(x)
    xs = np.ascontiguousarray(x.reshape(B, C, N)).astype(np.float32, copy=False)
    in_maps = [
        {
            "x": xs[b],
            "Wq": np.asarray(Wq, dtype=np.float32),
            "Wkv": np.asarray(Wkv, dtype=np.float32),
            "Wo": np.asarray(Wo, dtype=np.float32),
            "bo": np.asarray(bo, dtype=np.float32),
        }
        for b in range(B)
    ]
    res = run_bass_kernel_spmd(nc, in_maps, list(range(B)))
    return np.stack([res.results[b]["out"] for b in range(B)], axis=0)
